# revision 72
# baseline (speedup 1.0000x reference)
"""Trainium2 Bass kernel for LocalAttnLayer (sliding-window attention block).

Sharding: 8 cores = (batch b in 0..3) x (sequence half s in 0..1).
Each core processes 2048 tokens; the 128-token look-backward halo is handled
with a ring of per-window K/V tiles.

fp8 DoubleRow GEMMs with hi/lo error correction:
  All large GEMMs (QKV projections, FF1, FF2) run as fp8e4 (e4m3) matmuls in
  MatmulPerfMode.DoubleRow, which the PE executes at 0.5 cycles/row while
  contracting two 128-partition blocks per instruction.  Naive e4m3 (3-bit
  mantissa) breaks the 2e-2 gate, so operands are split hi/lo:
    X ~= Xhi + Xlo,  W ~= What + Wlo   (each part an e4m3 tensor)
  and each GEMM accumulates three DoubleRow products per pair of contraction
  blocks -- What*Xhi, What*Xlo, Wlo*Xhi -- i.e. 0.75x the f16 PE cost for
  QKV/FF1 (3-term).  FF2 uses 2-term (W2 hi/lo, h naive fp8): measured
  end-to-end rel err 1.3e-2 vs the 2e-2 gate.

Scale folding: weights are pre-scaled by 2^11 so their e4m3 encodings stay
normal.  The 2^11 is folded so K/V PSUM evictions are plain copies (on the
otherwise-idle Pool engine): K tiles hold 2^5*K (eviction scale 2^-6),
Q tiles hold Q/2^5 (epilogue scale 2^-16, keeping f16 normals), so
scores = K_t . Q_t are true-scale.  V tiles hold 2^11*V and the softmax
ones-column is memset to 2^11, so av/den is exact at the epilogue.

Attention itself (scores, exp, AV) stays f16: fp8 scores fail (softmax
amplification) and fp8 es fails (per-row dynamic range).

Per-core schedule (4 chunks x 512 tokens, window = 128): unchanged from the
f16 baseline -- P(n) projections / A(n) windowed attention with FF work
packed into the exp-wait bubbles, cross-chunk FF tails, hoisted next-chunk
Q projection, one LoadActFuncSet, f16 stores upcast on host.
"""

import os
import sys

for _p in ("/opt/trn_rl_repo", "/root/.axon_site/_ro/trn_rl_repo"):
    if os.path.isdir(_p) and _p not in sys.path:
        sys.path.insert(0, _p)

import numpy as np
import ml_dtypes

# Model dims (hardcoded per the problem spec)
B, S, D = 4, 4096, 1024
H, DH = 16, 64
WIN = 128
FH = 2048
LN_EPS = 1e-5

# Per-core sharding
T = 2048          # own tokens per core
HALO = 128
NCH = 4           # chunks per core
CT = 512          # tokens per chunk
CW = CT // WIN    # windows per chunk = 4
NW = NCH * CW     # windows per core = 16

EBT = -6.931471805599453  # exp bias: scores are within +-15, so
                          # exp(s + EBT) stays well inside f16 range

WS = 2.0 ** 11            # weight fp8 pre-scale
QSC = 2.0 ** -16          # Q epilogue scale  (Q_t = Q/2^5)
KSC = 2.0 ** -6           # K eviction scale  (K_t = 2^5*K)
FSC = 2.0 ** -11          # FF epilogue descale

_PROGRAM_CACHE = {}


def _build_program():
    import concourse.bass as bass
    import concourse.tile as tile
    from concourse import bacc, mybir
    from contextlib import ExitStack

    f8 = mybir.dt.float8e4
    f16 = mybir.dt.float16
    f32 = mybir.dt.float32
    AF = mybir.ActivationFunctionType
    ALU = mybir.AluOpType
    DR = mybir.MatmulPerfMode.DoubleRow

    # Pin the activation-table chooser to natural_log_exp_and_others (covers
    # exp, ln, identity, copy, relu) so the program needs exactly one
    # LoadActFuncSet.
    _orig_gat = bacc.get_activation_tables

    def _pinned_gat(arch, _orig=_orig_gat):
        keep = "natural_log_exp_and_others"
        full = dict(_orig(arch))
        if keep in full:
            return {k: (v if k == keep else set()) for k, v in full.items()}
        return full

    nc = bacc.Bacc("TRN2", target_bir_lowering=False, debug=False, num_devices=8)

    # ---- DRAM tensors (host pre-laid in SBUF tile layout) ----
    # xt8: [p, dt, hi/lo, col]; weight streams: [p, bi, 8, 2, 256] where
    # dim2 0:4 = What d-pairs, 4:8 = Wlo d-pairs and dim3 = pair member.
    xt8d = nc.dram_tensor("xt8", [128, 8, 2, HALO + T], f8, kind="ExternalInput").ap()
    xp = nc.dram_tensor("xp", [T, D], f16, kind="ExternalInput").ap()
    wq8 = nc.dram_tensor("wq8", [128, 4, 8, 2, 256], f8, kind="ExternalInput").ap()
    wk8 = nc.dram_tensor("wk8", [128, 4, 8, 2, 256], f8, kind="ExternalInput").ap()
    wv8 = nc.dram_tensor("wv8", [128, 4, 8, 2, 256], f8, kind="ExternalInput").ap()
    bqd = nc.dram_tensor("bq", [128, 8], f32, kind="ExternalInput").ap()
    w1d = nc.dram_tensor("w1", [128, 8, 2, FH], f8, kind="ExternalInput").ap()
    b1d = nc.dram_tensor("b1", [128, 16], f32, kind="ExternalInput").ap()
    w2d = nc.dram_tensor("w2", [128, 2, 16, D], f8, kind="ExternalInput").ap()
    b2d = nc.dram_tensor("b2", [1, D], f16, kind="ExternalInput").ap()
    g1d = nc.dram_tensor("g1", [D], f16, kind="ExternalInput").ap()
    g2d = nc.dram_tensor("g2", [D], f16, kind="ExternalInput").ap()
    bt2d = nc.dram_tensor("bt2", [D], f16, kind="ExternalInput").ap()
    cmkd = nc.dram_tensor("cmk", [WIN, WIN], f16, kind="ExternalInput").ap()
    m0d = nc.dram_tensor("m0", [128, 1], f32, kind="ExternalInput").ap()
    outd = nc.dram_tensor("out", [T, D], f16, kind="ExternalOutput").ap()

    xp_r = xp.rearrange("(n p) d -> n p d", p=128)
    out_r = outd.rearrange("(n p) d -> n p d", p=128)

    def bcast_ap(src_ap, parts=128):
        return bass.AP(
            tensor=src_ap.tensor,
            offset=src_ap.offset,
            ap=[[0, parts]] + [list(x) for x in src_ap.ap],
        )

    with tile.TileContext(nc) as tc, ExitStack() as ctx:
        # ---- pools ----
        singles = ctx.enter_context(tc.tile_pool(name="singles", bufs=1))
        w12_pool = ctx.enter_context(tc.tile_pool(name="w12", bufs=1))
        wblk_pool = ctx.enter_context(tc.tile_pool(name="wblk", bufs=3))
        xt_pool = ctx.enter_context(tc.tile_pool(name="xt", bufs=2))
        qt_pool = ctx.enter_context(tc.tile_pool(name="qt", bufs=2))
        kt_pool = ctx.enter_context(tc.tile_pool(name="kt", bufs=5))
        v_pool = ctx.enter_context(tc.tile_pool(name="v", bufs=5))
        es_pool = ctx.enter_context(tc.tile_pool(name="es", bufs=2))
        xp_pool = ctx.enter_context(tc.tile_pool(name="xpp", bufs=2))
        at_pool = ctx.enter_context(tc.tile_pool(name="at", bufs=2))
        tmp_pool = ctx.enter_context(tc.tile_pool(name="tmp", bufs=2))
        xhb_pool = ctx.enter_context(tc.tile_pool(name="xhb", bufs=6))
        xht_pool = ctx.enter_context(tc.tile_pool(name="xht", bufs=2))
        pk_pool = ctx.enter_context(tc.tile_pool(name="pk", bufs=1))
        ht_pool = ctx.enter_context(tc.tile_pool(name="ht", bufs=2))
        p2_pool = ctx.enter_context(tc.tile_pool(name="p2", bufs=2))
        oh_pool = ctx.enter_context(tc.tile_pool(name="oh", bufs=2))
        small = ctx.enter_context(tc.tile_pool(name="small", bufs=4))

        pp_ps = ctx.enter_context(tc.tile_pool(name="ppps", bufs=3, space="PSUM"))
        s_ps = ctx.enter_context(tc.tile_pool(name="sps", bufs=3, space="PSUM"))
        av_ps = ctx.enter_context(tc.tile_pool(name="avps", bufs=2, space="PSUM"))

        # ---- constants / broadcasts ----
        bq_sb = singles.tile([128, 8], f32)
        b1_sb = singles.tile([128, 16], f32)
        cm_b8 = singles.tile([WIN, 8, WIN], f16)
        m0_sb = singles.tile([128, 1], f32)
        g1b = singles.tile([128, D], f16)
        g2b = singles.tile([128, D], f16)
        b2b = singles.tile([128, D], f16)
        b2pb = singles.tile([128, D], f16)

        def load_consts():
            nc.sync.dma_start(
                out=cm_b8,
                in_=bass.AP(tensor=cmkd.tensor, offset=cmkd.offset,
                            ap=[list(cmkd.ap[0]), [0, 8], list(cmkd.ap[1])]),
            )
            nc.sync.dma_start(out=m0_sb, in_=m0d)
            nc.gpsimd.dma_start(out=g1b, in_=bcast_ap(g1d))
            nc.gpsimd.dma_start(out=g2b, in_=bcast_ap(g2d))
            nc.gpsimd.dma_start(out=b2b, in_=bcast_ap(bt2d))
            nc.gpsimd.dma_start(out=b2pb, in_=bcast_ap(b2d[0]))

        epst = singles.tile([128, 1], f32)
        nc.vector.memset(epst, LN_EPS)
        ebt = singles.tile([128, 1], f32)
        nc.vector.memset(ebt, EBT)
        # warm the exp activation table during the cold-start DMA wait so
        # chunk 0's first real exp doesn't pay the 1.28us table load
        atw = singles.tile([128, 1], f32)
        nc.scalar.activation(atw, ebt, AF.Exp)

        # ---- resident W1 / W2 (fp8 hi/lo) ----
        w1s = w12_pool.tile([128, 8, 2, FH], f8)
        w2s = w12_pool.tile([128, 2, 16, D], f8)

        def w12_piece(k):
            if k < 4:
                nc.sync.dma_start(
                    out=w1s[:, :, :, k * 512:(k + 1) * 512],
                    in_=w1d[:, :, :, k * 512:(k + 1) * 512],
                )
            else:
                k -= 4
                pl, hh = k // 2, (k % 2) * 8
                nc.sync.dma_start(
                    out=w2s[:, pl, hh:hh + 8, :],
                    in_=w2d[:, pl, hh:hh + 8, :],
                )

        # rings of per-window K/V tiles (index = global window, -1 = halo)
        ktiles = {}
        vtiles = {}

        def emit_proj_dr(ps_out, wb, xt_t, cols, tok, dsplit=False):
            """12 DoubleRow insts: What.Xhi, What.Xlo, Wlo.Xhi over 4 d-pairs.
            ps_out: PSUM AP slice; wb: weight tile [128,8,2,256]; cols: slice
            into wb's last dim; xt_t: [128,8,2,*]; tok: slice of token dim.
            dsplit: d-pair-ascending order so cold-start matmuls begin after
            the first 1-2 DMA pieces instead of the full block."""
            if dsplit:
                plan = []
                for t in range(4):
                    plan += [(t, 0, t), (t, 1, t)]
                plan += [(4 + t, 0, t) for t in range(4)]
            else:
                plan = [(t, 0, t) for t in range(4)] + \
                       [(t, 1, t) for t in range(4)] + \
                       [(4 + t, 0, t) for t in range(4)]
            n = len(plan)
            for i, (wt, xj, t) in enumerate(plan):
                nc.tensor.matmul(
                    ps_out, lhsT=wb[:, wt, :, cols],
                    rhs=xt_t[:, 2 * t:2 * t + 2, xj, tok],
                    start=(i == 0), stop=(i == n - 1), perf_mode=DR,
                )

        def proj_q_block(bi, xt_t, qt_t, wqb=None, split=False):
            if wqb is None:
                wqb = wblk_pool.tile([128, 8, 2, 256], f8, tag="wblk",
                                     name="wqb")
                nc.sync.dma_start(out=wqb, in_=wq8[:, bi])
            for sub in range(2):
                qc = bi * 2 + sub
                ps = pp_ps.tile([128, 512], f32, tag="pp")
                cols = slice(sub * 128, (sub + 1) * 128)
                emit_proj_dr(ps, wqb, xt_t, cols, slice(None), dsplit=split)
                if qc < 2:
                    nc.scalar.activation(
                        qt_t[:, qc, :], ps, AF.Identity,
                        bias=bq_sb[:, qc:qc + 1], scale=QSC,
                    )
                else:
                    # most qc blocks drain via DVE: the ACT queue carries the
                    # exps, so qt deps there make next-chunk scores wait
                    nc.vector.tensor_scalar(
                        qt_t[:, qc, :], ps, QSC, bq_sb[:, qc:qc + 1],
                        op0=ALU.mult, op1=ALU.add,
                    )

        def proj_k_block(bi, xt_t, xth_t, chn):
            """K projection for weight block bi (256 feature cols)."""
            wkb = wblk_pool.tile([128, 8, 2, 256], f8, tag="wblk")
            nc.sync.dma_start(out=wkb, in_=wk8[:, bi])
            for sub in range(2):
                kc = bi * 2 + sub
                cols = slice(sub * 128, (sub + 1) * 128)
                # K groups draw from the score PSUM pool: its previous
                # occupants drain via fast exps, not the chunk-tail FF1
                # relus that gate the pp pool at chunk boundaries
                ps = s_ps.tile([128, 512], f32, tag="s")
                emit_proj_dr(ps, wkb, xt_t, cols, slice(None))
                for w in range(CW):
                    # K eviction (PSUM->SBUF, 2^-6 fold); GPSIMD can't read
                    # PSUM, so alternate DVE/ACT by block parity to halve
                    # each queue's P-phase flood
                    if kc % 2 == 0:
                        nc.vector.tensor_scalar(
                            ktiles[chn * CW + w][:, kc, :],
                            ps[:, w * 128:(w + 1) * 128], KSC, None,
                            op0=ALU.mult,
                        )
                    else:
                        nc.scalar.activation(
                            ktiles[chn * CW + w][:, kc, :],
                            ps[:, w * 128:(w + 1) * 128], AF.Identity,
                            scale=KSC,
                        )
                if xth_t is not None:
                    ps2 = s_ps.tile([128, 512], f32, tag="s")
                    emit_proj_dr(ps2[:, 0:128], wkb, xth_t, cols, slice(None))
                    nc.vector.tensor_scalar(
                        ktiles[-1][:, kc, :], ps2[:, 0:128], KSC, None,
                        op0=ALU.mult,
                    )

        def emit_v_dr(ps_out, xt_t, wvb, tok):
            plan = [(0, t, t) for t in range(4)] + \
                   [(1, t, t) for t in range(4)] + \
                   [(0, 4 + t, t) for t in range(4)]
            n = len(plan)
            for i, (xj, wt, t) in enumerate(plan):
                nc.tensor.matmul(
                    ps_out, lhsT=xt_t[:, 2 * t:2 * t + 2, xj, tok],
                    rhs=wvb[:, wt, :, :],
                    start=(i == 0), stop=(i == n - 1), perf_mode=DR,
                )

        def proj_v_block(bi, xt_t, xth_t, chn):
            """V projection for weight block bi (256 feature cols = 4 heads)."""
            wvb = wblk_pool.tile([128, 8, 2, 256], f8, tag="wblk")
            nc.sync.dma_start(out=wvb, in_=wv8[:, bi])
            for w in range(CW):
                ps = pp_ps.tile([128, 512], f32, tag="pp")
                emit_v_dr(ps[:, 0:256], xt_t, wvb,
                          slice(w * 128, (w + 1) * 128))
                # V eviction, alternating DVE/ACT by block parity (the V
                # projection is emitted AFTER s_phase(0) so these queue
                # behind window 0's masks rather than ahead of its exps)
                vdst = vtiles[chn * CW + w][:, bi * 4:(bi + 1) * 4, 0:DH]
                vsrc = ps[:, 0:256].rearrange("p (h e) -> p h e", e=DH)
                if bi % 2 == 0:
                    nc.vector.tensor_copy(out=vdst, in_=vsrc)
                else:
                    nc.scalar.activation(vdst, vsrc, AF.Identity)
            if xth_t is not None:
                ps2 = pp_ps.tile([128, 512], f32, tag="pp")
                emit_v_dr(ps2[:, 0:256], xth_t, wvb, slice(None))
                nc.vector.tensor_copy(
                    out=vtiles[-1][:, bi * 4:(bi + 1) * 4, 0:DH],
                    in_=ps2[:, 0:256].rearrange("p (h e) -> p h e", e=DH),
                )
            return wvb

        FF1_PLAN = ([(t, 0, t) for t in range(4)] +
                    [(t, 1, t) for t in range(4)] +
                    [(4 + t, 0, t) for t in range(4)])

        def emit_ff1(chn, pair, xh8_list, ht_out, hcs=range(16)):
            """FF1 for window pair (256 tokens): ht_out[128, 16, 256] fp8."""
            n = len(FF1_PLAN)
            for hc in hcs:
                ps = pp_ps.tile([128, 512], f32, tag="pp")
                cols = slice(hc * 128, (hc + 1) * 128)
                for half in range(2):
                    xh8 = xh8_list[pair * 2 + half]
                    po = ps[:, half * 128:(half + 1) * 128]
                    for i, (wt, xj, t) in enumerate(FF1_PLAN):
                        nc.tensor.matmul(
                            po, lhsT=w1s[:, wt, :, cols],
                            rhs=xh8[:, 2 * t:2 * t + 2, xj, :],
                            start=(i == 0), stop=(i == n - 1), perf_mode=DR,
                        )
                nc.scalar.activation(
                    ht_out[:, hc, :], ps[:, 0:256], AF.Relu,
                    bias=b1_sb[:, hc:hc + 1], scale=FSC,
                )

        def emit_ff1_win(chn, pair, half, xh8_list, ht_out):
            """FF1 for ONE window of a pair (tail path: lets FF2(tile) start
            before the pair's second window exists)."""
            n = len(FF1_PLAN)
            xh8 = xh8_list[pair * 2 + half]
            for hc in range(16):
                ps = pp_ps.tile([128, 512], f32, tag="pp")
                cols = slice(hc * 128, (hc + 1) * 128)
                for i, (wt, xj, t) in enumerate(FF1_PLAN):
                    nc.tensor.matmul(
                        ps[:, 0:128], lhsT=w1s[:, wt, :, cols],
                        rhs=xh8[:, 2 * t:2 * t + 2, xj, :],
                        start=(i == 0), stop=(i == n - 1), perf_mode=DR,
                    )
                nc.scalar.activation(
                    ht_out[:, hc, half * 128:(half + 1) * 128],
                    ps[:, 0:128], AF.Relu,
                    bias=b1_sb[:, hc:hc + 1], scale=FSC,
                )

        def emit_ff2_ln2(chn, t, ht_t, tl, xhb_t, p2_t, store_q=None):
            """FF2 (2-term fp8 DR) + residual + LN2 + store for token tile t."""
            g = chn * CW + t
            st2 = small.tile([128, 2, 6], f32, tag="st2")
            tok = slice(tl * 128, (tl + 1) * 128)
            for yc in range(2):
                ps = pp_ps.tile([128, 512], f32, tag="pp")
                ycs = slice(yc * 512, (yc + 1) * 512)
                for i in range(16):
                    pl, tp = i // 8, i % 8
                    nc.tensor.matmul(
                        ps, lhsT=ht_t[:, 2 * tp:2 * tp + 2, tok],
                        rhs=w2s[:, pl, 2 * tp:2 * tp + 2, ycs],
                        start=(i == 0), stop=(i == 15), perf_mode=DR,
                    )
                nc.vector.scalar_tensor_tensor(
                    out=p2_t[:, ycs], in0=ps, scalar=FSC, in1=xhb_t[:, ycs],
                    op0=ALU.mult, op1=ALU.add,
                )
                nc.vector.bn_stats(out=st2[:, yc, :], in_=p2_t[:, ycs])
            mv2 = small.tile([128, 2], f32, tag="mv2")
            nc.vector.bn_aggr(out=mv2, in_=st2)
            lv2 = small.tile([128, 1], f32, tag="lv2")
            nc.scalar.activation(lv2, mv2[:, 1:2], AF.Ln, bias=epst)
            rstd2 = small.tile([128, 1], f32, tag="rstd2")
            nc.scalar.activation(rstd2, lv2, AF.Exp, scale=-0.5)
            nmr2 = small.tile([128, 1], f32, tag="nmr2")
            nc.vector.tensor_scalar(
                nmr2, mv2[:, 0:1], rstd2, -1.0, op0=ALU.mult, op1=ALU.mult
            )
            oh = oh_pool.tile([128, D], f16, tag="oh")
            if store_q is not None:
                for yc in range(2):
                    sl = slice(yc * 512, (yc + 1) * 512)
                    nc.scalar.activation(
                        oh[:, sl], p2_t[:, sl], AF.Identity,
                        bias=nmr2, scale=rstd2,
                    )
                    nc.vector.tensor_mul(oh[:, sl], oh[:, sl], g2b[:, sl])
                    nc.vector.tensor_add(oh[:, sl], oh[:, sl], b2b[:, sl])
                    store_q.dma_start(out=out_r[g][:, sl], in_=oh[:, sl])
            else:
                nc.scalar.activation(
                    oh, p2_t, AF.Identity, bias=nmr2, scale=rstd2)
                nc.vector.tensor_mul(oh, oh, g2b)
                nc.vector.tensor_add(oh, oh, b2b)
                nc.sync.dma_start(out=out_r[g], in_=oh)

        prev_tail = None
        hoisted_q = {}
        hoisted_xt = {}
        for chn in range(NCH):
            c0 = HALO + chn * CT

            # ---- x slab (fp8 hi/lo) for this chunk's own 512 tokens ----
            wqb0 = None
            if chn == 0:
                # cold start: d-pair-granular DMA interleave matches the
                # dsplit matmul order, so the first accumulation group's
                # inst t=0 starts after ~2.5KB of DMA instead of ~12KB
                wqb0 = wblk_pool.tile([128, 8, 2, 256], f8, tag="wblk",
                                      name="wqb")
                xt_t = xt_pool.tile([128, 8, 2, CT], f8, name="xt_t")
                for t in range(4):
                    nc.sync.dma_start(out=wqb0[:, t:t + 1], in_=wq8[:, 0, t:t + 1])
                    nc.sync.dma_start(out=xt_t[:, 2 * t:2 * t + 2],
                                      in_=xt8d[:, 2 * t:2 * t + 2, :, c0:c0 + CT])
                nc.sync.dma_start(out=wqb0[:, 4:8], in_=wq8[:, 0, 4:8])
                nc.sync.dma_start(out=bq_sb, in_=bqd)
                nc.sync.dma_start(out=b1_sb, in_=b1d)
            elif chn in hoisted_xt:
                xt_t = hoisted_xt.pop(chn)
            else:
                xt_t = xt_pool.tile([128, 8, 2, CT], f8, name="xt_t")
                nc.sync.dma_start(out=xt_t, in_=xt8d[:, :, :, c0:c0 + CT])
            xth_t = None
            if chn == 0:
                xth_t = xt_pool.tile([128, 8, 2, HALO], f8, tag="xth", bufs=1)
                ktiles[-1] = kt_pool.tile([128, 8, WIN], f16, tag="kt", name="kth")
                vtiles[-1] = v_pool.tile([128, H, DH + 1], f16, tag="vt", name="vth")
                nc.vector.memset(vtiles[-1][:, :, DH:DH + 1], WS)

            for w in range(CW):
                wg = chn * CW + w
                ktiles[wg] = kt_pool.tile([128, 8, WIN], f16, tag="kt", name="ktw")
                vtiles[wg] = v_pool.tile([128, H, DH + 1], f16, tag="vt", name="vtw")
                nc.vector.memset(vtiles[wg][:, :, DH:DH + 1], WS)

            # ---- P(n): projections ----
            if chn in hoisted_q:
                qt_t = hoisted_q.pop(chn)
            else:
                qt_t = qt_pool.tile([128, 8, CT], f16, name="qt_t")
                proj_q_block(0, xt_t, qt_t, wqb0, split=(chn == 0))
                for bi in range(1, 4):
                    proj_q_block(bi, xt_t, qt_t)
            if chn == 0:
                nc.sync.dma_start(out=xth_t, in_=xt8d[:, :, :, 0:HALO])
                load_consts()
            for bi in range(4):
                proj_k_block(bi, xt_t, xth_t, chn)
            # V projection is emitted after s_phase(0): its PE work fills
            # window 0's exp-wait bubble and its DVE evictions queue behind
            # window 0's masks instead of ahead of them.

            # hoist next chunk's Q projection into this chunk's attention
            # bubbles
            def load_q1(bi):
                wqb = wblk_pool.tile([128, 8, 2, 256], f8, tag="wblk",
                                     name="wqb")
                nc.sync.dma_start(out=wqb, in_=wq8[:, bi])
                return wqb

            # next-chunk hoist DMAs are emitted in the weave AFTER the
            # V-block loads so this chunk's V weights aren't stuck behind
            # next-chunk traffic on the in-order sync queue.  Only the first
            # two hoisted weight blocks load there; blocks 2-3 load
            # just-in-time (3-buf wblk pool: 4 at once deadlocks).
            q1_blocks = None
            xt1 = qt1 = None

            def emit_hoist(chn=chn):
                nc0 = HALO + (chn + 1) * CT
                xt1 = xt_pool.tile([128, 8, 2, CT], f8, name="xt_t")
                nc.sync.dma_start(out=xt1, in_=xt8d[:, :, :, nc0:nc0 + CT])
                blocks = [load_q1(0), load_q1(1)]
                qt1 = qt_pool.tile([128, 8, CT], f16, name="qt_t")
                hoisted_xt[chn + 1] = xt1
                hoisted_q[chn + 1] = qt1
                return xt1, qt1, blocks

            # ---- A(n) + interleaved F work ----
            xhb_list = {}
            xh8_list = {}
            ht_pair = {}
            p2_list = {}
            win_state = {}

            # score-group emission order (parity-alternating for the PSUM
            # lhsT-offset constraint); es slots are EMISSION-ordered so the
            # causal-mask multiplies batch over contiguous 8-slot ranges
            SEQ2 = (0, 4, 1, 5, 2, 6, 3, 7)
            SLOTH = []
            for _g2 in SEQ2:
                for _j in range(2):
                    _s = _g2 * 2 + _j
                    SLOTH.append(2 * _s if _s < 8 else 2 * (_s - 8) + 1)

            def s_phase(w, chn=chn, qt_t=qt_t):
                wg = chn * CW + w
                xpt = xp_pool.tile([128, D], f16, tag="xp")
                nc.sync.dma_start(out=xpt, in_=xp_r[wg])
                es = es_pool.tile([128, H, 2 * WIN], f16, tag="es")
                win_state[w] = (xpt, es)
                for idx, g2 in enumerate(SEQ2):
                    sps = s_ps.tile([128, 2, 2 * WIN], f32, tag="s")
                    off = 0 if g2 < 4 else 64
                    for j in range(2):
                        h = SLOTH[idx * 2 + j]
                        kprev = ktiles[wg - 1][off:off + 64, h // 2, :]
                        kcur = ktiles[wg][off:off + 64, h // 2, :]
                        qw = qt_t[off:off + 64, h // 2,
                                  w * 128:(w + 1) * 128]
                        nc.tensor.matmul(
                            sps[:, j, 0:128], lhsT=kprev, rhs=qw,
                            start=True, stop=True,
                        )
                        nc.tensor.matmul(
                            sps[:, j, 128:256], lhsT=kcur, rhs=qw,
                            start=True, stop=True,
                        )
                    sl = slice(idx * 2, (idx + 1) * 2)
                    nc.scalar.activation(es[:, sl, :], sps, AF.Exp, bias=ebt)
                    if idx % 4 == 3:
                        # one batched Pool mask per 4 exps (slots 8-contig);
                        # Pool is SBUF-only so it never queues behind the
                        # PSUM eviction floods on DVE/ACT
                        bs = slice((idx - 3) * 2, (idx + 1) * 2)
                        nc.gpsimd.tensor_mul(
                            es[:, bs, 128:256], es[:, bs, 128:256], cm_b8,
                        )
                        if wg == 0:
                            # first-window kprev mask is uniform per core
                            # (0 for seq-half 0, 1 for half 1): scalar mult
                            nc.gpsimd.tensor_scalar(
                                es[:, bs, 0:128], es[:, bs, 0:128],
                                m0_sb[:, 0:1], None, op0=ALU.mult,
                            )

            def av_phase(w, chn=chn):
                wg = chn * CW + w
                xpt, es = win_state.pop(w)
                at_t = at_pool.tile([128, D], f16, tag="at")
                for g in range(4):
                    av = av_ps.tile([128, 4, 128], f32, tag="av")
                    for j in range(4):
                        s = g * 4 + j
                        h = SLOTH[s]
                        nc.tensor.matmul(
                            av[:, j, 0:DH + 1], lhsT=es[:, s, 0:128],
                            rhs=vtiles[wg - 1][:, h, :],
                            start=True, stop=False,
                        )
                        nc.tensor.matmul(
                            av[:, j, 0:DH + 1], lhsT=es[:, s, 128:256],
                            rhs=vtiles[wg][:, h, :],
                            start=False, stop=True,
                        )
                    rden = small.tile([128, 4], f32, tag="rden")
                    nc.vector.reciprocal(rden, av[:, :, DH:DH + 1])
                    for j in range(4):
                        s = g * 4 + j
                        h = SLOTH[s]
                        nc.vector.scalar_tensor_tensor(
                            out=at_t[:, h * DH:(h + 1) * DH],
                            in0=av[:, j, 0:DH],
                            scalar=rden[:, j:j + 1],
                            in1=xpt[:, h * DH:(h + 1) * DH],
                            op0=ALU.mult, op1=ALU.add,
                        )
                # LN1
                stats = small.tile([128, 2, 6], f32, tag="st1")
                atv = at_t.rearrange("p (a b) -> p a b", b=512)
                for sg in range(2):
                    nc.vector.bn_stats(out=stats[:, sg, :], in_=atv[:, sg, :])
                mv = small.tile([128, 2], f32, tag="mv1")
                nc.vector.bn_aggr(out=mv, in_=stats)
                lv = small.tile([128, 1], f32, tag="lv1")
                nc.scalar.activation(lv, mv[:, 1:2], AF.Ln, bias=epst)
                rstd = small.tile([128, 1], f32, tag="rstd1")
                nc.scalar.activation(rstd, lv, AF.Exp, scale=-0.5)
                nmr = small.tile([128, 1], f32, tag="nmr1")
                nc.vector.tensor_scalar(
                    nmr, mv[:, 0:1], rstd, -1.0, op0=ALU.mult, op1=ALU.mult
                )
                # x-hat hi/lo fp8 packed into byte planes of one f16-element
                # tile BEFORE the transpose: the XBAR moves the (hi,lo) pair
                # as one 2-byte word, so the transposed tile is directly the
                # FF1 DoubleRow operand (no post-transpose quantize ops).
                tmp = tmp_pool.tile([128, D], f16, tag="tmp")
                nc.vector.tensor_scalar(
                    tmp, at_t, rstd, nmr, op0=ALU.mult, op1=ALU.add)
                pack = pk_pool.tile([128, D], f16, tag="pack")
                pk8 = pack[:, :].bitcast(f8).rearrange(
                    "p (d two) -> p d two", two=2)
                nc.scalar.activation(
                    pk8[:, :, 0], at_t, AF.Identity, bias=nmr, scale=rstd)
                nc.vector.tensor_sub(pk8[:, :, 1], tmp, pk8[:, :, 0])
                xht = xht_pool.tile([128, 8, WIN], f16, tag="xht")
                nc.scalar.dma_start_transpose(xht, pack)
                xh8_list[w] = xht[:, :, :].bitcast(f8).rearrange(
                    "p e (t two) -> p e two t", two=2)
                # xhb = xhat*g1 + b2p on the Pool engine (SBUF-only, and its
                # consumer FF2 is 1-2 windows away so the slow Pool is fine)
                xhb = xhb_pool.tile([128, D], f16, tag="xhb")
                nc.gpsimd.tensor_mul(xhb, tmp, g1b)
                nc.gpsimd.tensor_add(xhb, xhb, b2pb)
                xhb_list[w] = xhb

            def make_ff1(pair, chn=chn, xh8_list=xh8_list, ht_pair=ht_pair,
                         half=None):
                def run():
                    if half in (None, 0):
                        ht_pair[pair] = ht_pool.tile(
                            [128, 16, 2 * WIN], f8, tag="ht", name="htp"
                        )
                    hcs = (range(16) if half is None else
                           range(8) if half == 0 else range(8, 16))
                    emit_ff1(chn, pair, xh8_list, ht_pair[pair], hcs)
                return run

            def make_ff1_win(pair, win, chn=chn, xh8_list=xh8_list,
                             ht_pair=ht_pair):
                def run():
                    if win == 0:
                        ht_pair[pair] = ht_pool.tile(
                            [128, 16, 2 * WIN], f8, tag="ht", name="htp"
                        )
                    emit_ff1_win(chn, pair, win, xh8_list, ht_pair[pair])
                return run

            def make_ff2(t, chn=chn, xhb_list=xhb_list, ht_pair=ht_pair,
                         p2_list=p2_list):
                def run():
                    p2_list[t] = p2_pool.tile([128, D], f16, tag="p2",
                                              name="p2t")
                    sq = nc.scalar if (chn == NCH - 1 and t >= 2) else None
                    emit_ff2_ln2(chn, t, ht_pair[t // 2], t % 2,
                                 xhb_list[t], p2_list[t], store_q=sq)
                return run

            # 2-ahead weave: s(w+1) is emitted before av(w) so the PE always
            # has independent score/projection work while av(w) waits on the
            # exp->mask chain.
            s_phase(0)
            if chn == 0:
                for k in range(4):
                    w12_piece(k)
            for bi in range(4):
                proj_v_block(bi, xt_t, xth_t, chn)
            if chn < NCH - 1:
                xt1, qt1, q1_blocks = emit_hoist()
            if q1_blocks:
                proj_q_block(0, xt1, qt1, q1_blocks[0])
            s_phase(1)
            if chn == 0:
                for k in range(4, 8):
                    w12_piece(k)
            if prev_tail:
                # deferred prev-chunk FF1 sits AFTER this chunk's scores,
                # V projection and hoisted-Q in the PE stream, so its xh8
                # waits are covered by independent matmul work
                prev_tail[0]()
            if q1_blocks:
                proj_q_block(1, xt1, qt1, q1_blocks[1])
                q1_blocks.append(load_q1(2))
            if prev_tail:
                prev_tail[1]()
            av_phase(0)
            if prev_tail:
                prev_tail[2]()
            if q1_blocks:
                q1_blocks.append(load_q1(3))
            s_phase(2)
            if q1_blocks:
                proj_q_block(2, xt1, qt1, q1_blocks[2])
            av_phase(1)
            if q1_blocks:
                proj_q_block(3, xt1, qt1, q1_blocks[3])
            if prev_tail:
                prev_tail[3]()
            s_phase(3)
            make_ff1(0, half=0)()
            av_phase(2)
            make_ff1(0, half=1)()
            make_ff2(0)()
            if chn == NCH - 1:
                # tail: window-granular FF1(pair1) so FF2(t2) runs off
                # window 2's xh8 alone instead of waiting for av3
                ht_pair[1] = ht_pool.tile([128, 16, 2 * WIN], f8, tag="ht",
                                          name="htp")
                emit_ff1_win(chn, 1, 0, xh8_list, ht_pair[1])
                av_phase(3)
                make_ff2(1)()
                make_ff2(2)()
                emit_ff1_win(chn, 1, 1, xh8_list, ht_pair[1])
                make_ff2(3)()
            else:
                av_phase(3)
                make_ff2(1)()
                prev_tail = [make_ff1(1, half=0), make_ff1(1, half=1),
                             make_ff2(2), make_ff2(3)]

    bacc.get_activation_tables = _pinned_gat
    try:
        nc.compile()
    finally:
        bacc.get_activation_tables = _orig_gat
    return nc


def _get_program():
    if "nc" not in _PROGRAM_CACHE:
        _PROGRAM_CACHE["nc"] = _build_program()
    return _PROGRAM_CACHE["nc"]


def _f8(a):
    return a.astype(ml_dtypes.float8_e4m3)


def _pack_w_pairs(Wmat):
    """[D_in, N] f32 (pre-scaled) -> [128, 8, 2, N] fp8: dim1 0:4 = What
    d-pairs, 4:8 = Wlo d-pairs; dim2 = pair member (d = (2t+i)*128+p)."""
    hat = _f8(Wmat)
    lo = _f8(Wmat - hat.astype(np.float32))

    def lay(a):
        return a.reshape(4, 2, 128, a.shape[-1]).transpose(2, 0, 1, 3)

    return np.ascontiguousarray(
        np.concatenate([lay(hat), lay(lo)], axis=1))


def _block_w(packed):
    """[128, 8, 2, D(=1024)] -> [128, 4bi, 8, 2, 256] for streamed loads."""
    p, e, i, n = packed.shape
    return np.ascontiguousarray(
        packed.reshape(p, e, i, 4, 256).transpose(0, 3, 1, 2, 4))


def make_in_maps(x, q_proj, k_proj, v_proj, q_bias, k_bias, v_bias,
                 ln1_g, ln1_b, ln2_g, ln2_b, ff1_w, ff1_b, ff2_w, ff2_b):
    """Host-side prep: fold biases/scales, fp8 hi/lo split, shard 8 cores."""
    x = np.asarray(x, np.float32)
    scale = DH ** -0.5

    Wq = (np.transpose(np.asarray(q_proj, np.float32), (1, 0, 2)).reshape(D, D)
          * scale)
    Wk = np.transpose(np.asarray(k_proj, np.float32), (1, 0, 2)).reshape(D, D)
    Wv = np.transpose(np.asarray(v_proj, np.float32), (1, 0, 2)).reshape(D, D)
    bq_full = (np.asarray(q_bias, np.float32).reshape(D) * scale)
    bv_full = np.asarray(v_bias, np.float32).reshape(D)

    ln1_g = np.asarray(ln1_g, np.float32)
    ln1_b = np.asarray(ln1_b, np.float32)
    ff1_w = np.asarray(ff1_w, np.float32)
    ff1_b = np.asarray(ff1_b, np.float32)
    ff2_w = np.asarray(ff2_w, np.float32)
    ff2_b = np.asarray(ff2_b, np.float32)

    wq8 = _block_w(_pack_w_pairs(Wq * WS))
    wk8 = _block_w(_pack_w_pairs(Wk * WS))
    wv8 = _block_w(_pack_w_pairs(Wv * WS))

    W1 = ff1_w * ln1_g[:, None]
    b1_full = ff1_b + ln1_b @ ff1_w
    b2_full = (ff2_b + ln1_b).astype(np.float16)
    w1_8 = _pack_w_pairs(W1 * WS)

    W2s = ff2_w * WS
    w2hat = _f8(W2s)
    w2lo = _f8(W2s - w2hat.astype(np.float32))
    # [FH, D] -> [128, 16, D] (hid = hc*128+p), then stack hat/lo planes
    def layw2(a):
        return a.reshape(16, 128, D).transpose(1, 0, 2)
    w2_8 = np.ascontiguousarray(
        np.stack([layw2(w2hat), layw2(w2lo)], axis=1))

    bq_l = np.ascontiguousarray(
        (bq_full / 32.0).reshape(8, 128).T.astype(np.float32))
    b1_l = np.ascontiguousarray(b1_full.reshape(16, 128).T.astype(np.float32))

    kq = np.arange(WIN)
    cmk = (kq[None, :] >= kq[:, None]).astype(np.float16)

    common = {
        "wq8": wq8, "wk8": wk8, "wv8": wv8,
        "bq": bq_l, "w1": w1_8, "b1": b1_l, "w2": w2_8,
        "b2": b2_full.reshape(1, D),
        "g1": ln1_g.astype(np.float16),
        "g2": np.asarray(ln2_g, np.float16),
        "bt2": np.asarray(ln2_b, np.float16),
        "cmk": cmk,
    }

    in_maps = []
    for b in range(B):
        for s in range(2):
            own = x[b, s * T:(s + 1) * T]
            if s == 0:
                halo = np.zeros((HALO, D), np.float32)
                m0 = np.zeros((128, 1), np.float32)
            else:
                halo = x[b, s * T - HALO:s * T]
                m0 = np.ones((128, 1), np.float32)
            xTs = np.concatenate([halo, own], axis=0).T  # [D, HALO+T]
            xhi = _f8(xTs)
            xlo = _f8(xTs - xhi.astype(np.float32))
            C = HALO + T
            xt8 = np.ascontiguousarray(np.stack(
                [xhi.reshape(8, 128, C).transpose(1, 0, 2),
                 xlo.reshape(8, 128, C).transpose(1, 0, 2)], axis=2))
            xpa = (own + bv_full[None, :]).astype(np.float16)
            in_maps.append({
                **common,
                "xt8": xt8,
                "xp": np.ascontiguousarray(xpa),
                "m0": m0,
            })
    return in_maps


def gather_outputs(results):
    out = np.empty((B, S, D), np.float32)
    for b in range(B):
        for s in range(2):
            out[b, s * T:(s + 1) * T] = results[b * 2 + s]["out"].astype(np.float32)
    return out


def kernel(**inputs):
    from concourse import bass_utils

    nc = _get_program()
    in_maps = make_in_maps(**inputs)
    res = bass_utils.run_bass_kernel_spmd(nc, in_maps, core_ids=list(range(8)))
    return gather_outputs(res.results)
